# revision 9
# baseline (speedup 1.0000x reference)
"""GIN message-passing (2 GINConv layers + 2 linear) on 8 TRN2 NeuronCores.

Strategy (self-contained; shapes hardcoded for the 100k-node / 1.6M-edge
problem):
  - Shard dst nodes across 8 cores (12500 each). Each core owns the edges
    whose dst is in its shard.
  - Per core, dst tiles of 128 nodes. For each tile, gather the source-node
    feature rows with gpsimd.dma_gather (int16 indices -> split the node
    table into 4 quarters of 25000 rows), then aggregate with a one-hot
    matmul on the tensor engine: aggT[ch, dst] += Xe[slots, ch].T @ onehot.
  - Layer 1 gathers x in fp16 (256B rows) straight from the replicated
    input; it computes z = relu((x + A@x)@W1 + b1) @ W2 per shard, writes z
    row-major fp32 to HBM, and an AllGather shares z across cores.
  - Layer 2 gathers z (fp32 256B rows), then h2 = relu(z_dst + A@z + b2),
    h3 = relu(h2@W3+b3), out = h3@W4+b4.
  - Gathers are batched over T_B dst tiles per call to amortize the ~1us
    SWDGE fixed cost; per-(tile,quarter) budgets are static (max over
    cores, rounded to 16) with idx-0 padding masked by dstloc=-1 one-hots.
"""

import numpy as np

P = 128


class Cfg:
    def __init__(self, n_nodes, n_edges, in_ch, hid, n_cores, t_b):
        assert n_nodes % (4 * n_cores) == 0
        self.N = n_nodes
        self.E = n_edges
        self.CH = in_ch
        self.H = hid
        self.NCORE = n_cores
        self.SHARD = n_nodes // n_cores
        self.NT = -(-self.SHARD // P)
        self.NQ = 4
        self.QS = n_nodes // 4
        assert self.QS <= 32768
        self.T_B = t_b
        self.NB = -(-self.NT // t_b)


FULL = Cfg(100000, 1600000, 128, 64, 8, 4)


def _r16(a):
    return ((a + 15) // 16) * 16


def _r128(a):
    return ((a + 127) // 128) * 128


class Plan:
    """Static (core-independent) schedule + per-core index arrays."""

    def __init__(self, cfg, src, dst):
        c = cfg
        self.cfg = c
        core = dst // c.SHARD
        # per-core grouped edge arrays
        self.counts = np.zeros((c.NCORE, c.NT, c.NQ), dtype=np.int64)
        self.groups = []  # [core] -> dict[(t,q)] -> (srclocal i16 array, dstlocal array)
        for ci in range(c.NCORE):
            m = core == ci
            s = src[m]
            d = dst[m] - ci * c.SHARD
            t = d // P
            q = s // c.QS
            key = t * c.NQ + q
            order = np.argsort(key, kind="stable")
            s, d, t, q, key = s[order], d[order], t[order], q[order], key[order]
            sl = (s % c.QS).astype(np.int16)
            dl = (d % P).astype(np.int16)
            cnt = np.bincount(key, minlength=c.NT * c.NQ).reshape(c.NT, c.NQ)
            self.counts[ci] = cnt
            bounds = np.concatenate([[0], np.cumsum(cnt.reshape(-1))])
            g = {}
            for tt in range(c.NT):
                for qq in range(c.NQ):
                    k = tt * c.NQ + qq
                    lo, hi = bounds[k], bounds[k + 1]
                    if hi > lo:
                        g[(tt, qq)] = (sl[lo:hi], dl[lo:hi])
            self.groups.append(g)

        # x128 so every tile's segment is chunk-aligned (PE matmuls need
        # base partition 0) and every matmul has full K=128
        self.B = _r128(self.counts.max(axis=0))  # [NT, NQ] slot budgets
        # per (batch, quarter): total slots, chunk layout, segments
        self.batches = []
        cgo = 0  # global chunk offset (dstloc columns)
        igo = 0  # global idx16 column offset
        for b in range(c.NB):
            tiles = list(range(b * c.T_B, min((b + 1) * c.T_B, c.NT)))
            quarters = []
            o = 0  # chunk offset within batch
            for q in range(c.NQ):
                segs = []  # (tile, chunk_in_q, p0, p1, slot0_of_tile_seg)
                s0 = 0
                for t in tiles:
                    btq = int(self.B[t, q])
                    if btq == 0:
                        continue
                    lo, hi = s0, s0 + btq
                    c0, c1 = lo // P, (hi - 1) // P
                    for ch in range(c0, c1 + 1):
                        p0 = max(lo, ch * P) - ch * P
                        p1 = min(hi, (ch + 1) * P) - ch * P
                        segs.append((t, ch, p0, p1))
                    s0 = hi
                tot = s0  # multiple of 16
                nch = -(-tot // P) if tot else 0
                quarters.append(
                    dict(tot=tot, nch=nch, o=o, icols=tot // 16, segs=segs)
                )
                o += nch
            self.batches.append(
                dict(tiles=tiles, quarters=quarters, cgo=cgo, igo=igo, nch=o)
            )
            cgo += o
            igo += sum(qq["icols"] for qq in quarters)
        self.CGOT = cgo
        self.IGOT = igo

    def core_arrays(self, ci):
        """idx16 [128, IGOT] int16 and dstloc [128, CGOT] f16 for core ci."""
        c = self.cfg
        idx16 = np.zeros((P, self.IGOT), dtype=np.int16)
        dstloc = np.full((P, self.CGOT), -1.0, dtype=np.float16)
        g = self.groups[ci]
        for b in self.batches:
            icol = b["igo"]
            for q, qq in enumerate(b["quarters"]):
                tot = qq["tot"]
                if tot == 0:
                    continue
                sl_full = np.zeros(tot, dtype=np.int16)
                dl_full = np.full(tot, -1, dtype=np.int16)
                s0 = 0
                for t in b["tiles"]:
                    btq = int(self.B[t, q])
                    if btq == 0:
                        continue
                    if (t, q) in g:
                        sl, dl = g[(t, q)]
                        n = len(sl)
                        sl_full[s0 : s0 + n] = sl
                        dl_full[s0 : s0 + n] = dl
                    s0 += btq
                # idx wrap-16 layout, replicated to 128 partitions
                blk = sl_full.reshape(tot // 16, 16).T  # [16, tot/16]
                idx16[:, icol : icol + tot // 16] = np.tile(blk, (8, 1))
                icol += tot // 16
                # dstloc slot k -> [k%128, cgo + o + k//128]
                col0 = b["cgo"] + qq["o"]
                nch = qq["nch"]
                dpad = np.full(nch * P, -1, dtype=np.int16)
                dpad[:tot] = dl_full
                dstloc[:, col0 : col0 + nch] = (
                    dpad.reshape(nch, P).T.astype(np.float16)
                )
        return idx16, dstloc


def _build(plan):
    import concourse.bass as bass
    import concourse.tile as tile
    from concourse import bacc, mybir

    c = plan.cfg
    f16, f32, i16 = mybir.dt.float16, mybir.dt.float32, mybir.dt.int16
    CH, H, NT, NQ, QS, SHARD = c.CH, c.H, c.NT, c.NQ, c.QS, c.SHARD

    nc = bacc.Bacc(
        "TRN2", target_bir_lowering=False, debug=False, num_devices=c.NCORE
    )
    x16 = nc.dram_tensor("x16", [c.N, CH], f16, kind="ExternalInput")
    xT = nc.dram_tensor("xT", [CH, SHARD], f32, kind="ExternalInput")
    pk = nc.dram_tensor(
        "pk", [P, plan.IGOT + plan.CGOT], i16, kind="ExternalInput"
    )
    iota = nc.dram_tensor("iota", [P, P], f16, kind="ExternalInput")
    ident = nc.dram_tensor("ident", [H, H], f16, kind="ExternalInput")
    w1 = nc.dram_tensor("w1", [CH, H], f16, kind="ExternalInput")
    w2 = nc.dram_tensor("w2", [H, H], f16, kind="ExternalInput")
    w3 = nc.dram_tensor("w3", [H, 16], f16, kind="ExternalInput")
    w4 = nc.dram_tensor("w4", [16, 1], f16, kind="ExternalInput")
    b1 = nc.dram_tensor("b1", [H, 1], f32, kind="ExternalInput")
    b2 = nc.dram_tensor("b2", [H, 1], f32, kind="ExternalInput")
    b3 = nc.dram_tensor("b3", [16, 1], f32, kind="ExternalInput")
    b4v = nc.dram_tensor("b4v", [1, 1], f32, kind="ExternalInput")
    out = nc.dram_tensor("out", [1, SHARD], f32, kind="ExternalOutput")

    with tile.TileContext(nc) as tc:
        with (
            tc.tile_pool(name="const", bufs=1) as cp,
            tc.tile_pool(name="persist", bufs=1) as pp,
            tc.tile_pool(name="dram", bufs=1, space="DRAM") as dp,
        ):
            iota_sb = cp.tile([P, P], f16)
            nc.sync.dma_start(out=iota_sb[:], in_=iota[:, :])
            id_sb = cp.tile([H, H], f16)
            nc.sync.dma_start(out=id_sb[:], in_=ident[:, :])
            w1_sb = cp.tile([CH, H], f16)
            nc.sync.dma_start(out=w1_sb[:], in_=w1[:, :])
            w2_sb = cp.tile([H, H], f16)
            nc.sync.dma_start(out=w2_sb[:], in_=w2[:, :])
            w3_sb = cp.tile([H, 16], f16)
            nc.sync.dma_start(out=w3_sb[:], in_=w3[:, :])
            w4_sb = cp.tile([16, 1], f16)
            nc.sync.dma_start(out=w4_sb[:], in_=w4[:, :])
            b1_sb = cp.tile([H, 1], f32)
            nc.sync.dma_start(out=b1_sb[:], in_=b1[:, :])
            b2_sb = cp.tile([H, 1], f32)
            nc.sync.dma_start(out=b2_sb[:], in_=b2[:, :])
            b3_sb = cp.tile([16, 1], f32)
            nc.sync.dma_start(out=b3_sb[:], in_=b3[:, :])
            b4_sb = cp.tile([1, 1], f32)
            nc.sync.dma_start(out=b4_sb[:], in_=b4v[:, :])

            zT_sh = pp.tile([H, SHARD], f32)  # identity term for layer 2
            outT = pp.tile([1, SHARD], f32)

            z_shard = dp.tile([SHARD, H], f32)
            z_full = dp.tile([c.N, H], f32)

            relu = mybir.ActivationFunctionType.Relu

            def tile_cols(t):
                return min(P, SHARD - t * P)

            # ---------------- phase 1 ----------------
            with (
                tc.tile_pool(name="l1g", bufs=2) as gp,
                tc.tile_pool(name="l1oh", bufs=2) as ohp,
                tc.tile_pool(name="l1sm", bufs=3) as sm,
                tc.tile_pool(name="l1ps", bufs=c.T_B + 1, space="PSUM") as psa,
                tc.tile_pool(name="l1pst", bufs=1, space="PSUM") as pst,
            ):
                for b in plan.batches:
                    nch = b["nch"]
                    if nch == 0:
                        continue
                    icols = sum(q["icols"] for q in b["quarters"])
                    pk_sb = sm.tile([P, icols + nch], i16, tag="pk")
                    nc.sync.dma_start(
                        out=pk_sb[:, :icols],
                        in_=pk[:, b["igo"] : b["igo"] + icols],
                    )
                    nc.sync.dma_start(
                        out=pk_sb[:, icols:],
                        in_=pk[
                            :,
                            plan.IGOT + b["cgo"] : plan.IGOT + b["cgo"] + nch,
                        ],
                    )
                    dl_sb = pk_sb[:, icols:].bitcast(f16)
                    g1 = gp.tile([P, nch, CH], f16, tag="g1")
                    ic = 0
                    for q, qq in enumerate(b["quarters"]):
                        if qq["tot"] == 0:
                            continue
                        # dma_gather is limited to 1024 indices per call
                        for s0 in range(0, qq["tot"], 1024):
                            n = min(1024, qq["tot"] - s0)
                            c0 = qq["o"] + s0 // P
                            nc.gpsimd.dma_gather(
                                g1[:, c0 : c0 + n // P, :],
                                x16[q * QS : (q + 1) * QS, :],
                                pk_sb[:, ic + s0 // 16 : ic + (s0 + n) // 16],
                                n,
                                n,
                                CH,
                            )
                        ic += qq["icols"]
                    oh = ohp.tile([P, nch, P], f16, tag="oh")
                    nc.vector.tensor_tensor(
                        out=oh[:],
                        in0=dl_sb.unsqueeze(2).to_broadcast([P, nch, P]),
                        in1=iota_sb[:].unsqueeze(1).to_broadcast([P, nch, P]),
                        op=mybir.AluOpType.is_equal,
                    )
                    aggs = {}
                    mm = []
                    for q, qq in enumerate(b["quarters"]):
                        for (t, ch, p0, p1) in qq["segs"]:
                            mm.append((t, qq["o"] + ch, p0, p1))
                    first = {}
                    last = {}
                    for i, (t, ch, p0, p1) in enumerate(mm):
                        first.setdefault(t, i)
                        last[t] = i
                    for t in b["tiles"]:
                        aggs[t] = psa.tile([CH, P], f32, tag="agg1", name=f"agg1_{t}")
                    for i, (t, ch, p0, p1) in enumerate(mm):
                        nc.tensor.matmul(
                            out=aggs[t][:],
                            lhsT=g1[p0:p1, ch : ch + 1, :],
                            rhs=oh[p0:p1, ch : ch + 1, :],
                            start=(i == first[t]),
                            stop=(i == last[t]),
                        )
                    for t in b["tiles"]:
                        tw = tile_cols(t)
                        xT_sb = sm.tile([CH, P], f32, tag="xt")
                        nc.scalar.dma_start(
                            out=xT_sb[:, :tw], in_=xT[:, t * P : t * P + tw]
                        )
                        sT = sm.tile([CH, P], f16, tag="st")
                        nc.vector.tensor_add(
                            out=sT[:, :tw],
                            in0=aggs[t][:, :tw],
                            in1=xT_sb[:, :tw],
                        )
                        h1p = pst.tile([H, P], f32, tag="h1")
                        nc.tensor.matmul(
                            out=h1p[:, :tw], lhsT=w1_sb[:], rhs=sT[:, :tw],
                            start=True, stop=True,
                        )
                        h1f = sm.tile([H, P], f16, tag="h1f")
                        nc.scalar.activation(
                            out=h1f[:, :tw], in_=h1p[:, :tw], func=relu,
                            bias=b1_sb[:],
                        )
                        zp = pst.tile([H, P], f32, tag="zp")
                        nc.tensor.matmul(
                            out=zp[:, :tw], lhsT=w2_sb[:], rhs=h1f[:, :tw],
                            start=True, stop=True,
                        )
                        nc.vector.tensor_copy(
                            out=zT_sh[:, t * P : t * P + tw], in_=zp[:, :tw]
                        )
                        zf = sm.tile([H, P], f16, tag="zf")
                        nc.vector.tensor_copy(out=zf[:, :tw], in_=zp[:, :tw])
                        ztr = pst.tile([P, H], f16, tag="ztr")
                        nc.tensor.transpose(
                            out=ztr[:tw, :], in_=zf[:, :tw], identity=id_sb[:]
                        )
                        zr = sm.tile([P, H], f32, tag="zr")
                        nc.vector.tensor_copy(out=zr[:tw, :], in_=ztr[:tw, :])
                        nc.scalar.dma_start(
                            out=z_shard[t * P : t * P + tw, :], in_=zr[:tw, :]
                        )

            nc.gpsimd.collective_compute(
                "AllGather",
                mybir.AluOpType.bypass,
                replica_groups=[list(range(c.NCORE))],
                ins=[z_shard[:].opt()],
                outs=[z_full[:].opt()],
            )

            # ---------------- phase 2 ----------------
            with (
                tc.tile_pool(name="l2g", bufs=2) as gp,
                tc.tile_pool(name="l2oh", bufs=2) as ohp,
                tc.tile_pool(name="l2sm", bufs=3) as sm,
                tc.tile_pool(name="l2ps", bufs=c.T_B + 1, space="PSUM") as psa,
                tc.tile_pool(name="l2pst", bufs=1, space="PSUM") as pst,
            ):
                for b in plan.batches:
                    nch = b["nch"]
                    if nch == 0:
                        continue
                    icols = sum(q["icols"] for q in b["quarters"])
                    pk_sb = sm.tile([P, icols + nch], i16, tag="pk")
                    nc.sync.dma_start(
                        out=pk_sb[:, :icols],
                        in_=pk[:, b["igo"] : b["igo"] + icols],
                    )
                    nc.sync.dma_start(
                        out=pk_sb[:, icols:],
                        in_=pk[
                            :,
                            plan.IGOT + b["cgo"] : plan.IGOT + b["cgo"] + nch,
                        ],
                    )
                    dl_sb = pk_sb[:, icols:].bitcast(f16)
                    g2 = gp.tile([P, nch, H], f32, tag="g2")
                    ic = 0
                    for q, qq in enumerate(b["quarters"]):
                        if qq["tot"] == 0:
                            continue
                        for s0 in range(0, qq["tot"], 1024):
                            n = min(1024, qq["tot"] - s0)
                            c0 = qq["o"] + s0 // P
                            nc.gpsimd.dma_gather(
                                g2[:, c0 : c0 + n // P, :],
                                z_full[q * QS : (q + 1) * QS, :],
                                pk_sb[:, ic + s0 // 16 : ic + (s0 + n) // 16],
                                n,
                                n,
                                H,
                            )
                        ic += qq["icols"]
                    g2f = gp.tile([P, nch, H], f16, tag="g2f")
                    nc.vector.tensor_copy(out=g2f[:], in_=g2[:])
                    oh = ohp.tile([P, nch, P], f16, tag="oh")
                    nc.vector.tensor_tensor(
                        out=oh[:],
                        in0=dl_sb.unsqueeze(2).to_broadcast([P, nch, P]),
                        in1=iota_sb[:].unsqueeze(1).to_broadcast([P, nch, P]),
                        op=mybir.AluOpType.is_equal,
                    )
                    mm = []
                    for q, qq in enumerate(b["quarters"]):
                        for (t, ch, p0, p1) in qq["segs"]:
                            mm.append((t, qq["o"] + ch, p0, p1))
                    first = {}
                    last = {}
                    for i, (t, ch, p0, p1) in enumerate(mm):
                        first.setdefault(t, i)
                        last[t] = i
                    aggs = {}
                    for t in b["tiles"]:
                        aggs[t] = psa.tile([H, P], f32, tag="agg2", name=f"agg2_{t}")
                    for i, (t, ch, p0, p1) in enumerate(mm):
                        nc.tensor.matmul(
                            out=aggs[t][:],
                            lhsT=g2f[p0:p1, ch : ch + 1, :],
                            rhs=oh[p0:p1, ch : ch + 1, :],
                            start=(i == first[t]),
                            stop=(i == last[t]),
                        )
                    for t in b["tiles"]:
                        tw = tile_cols(t)
                        t2 = sm.tile([H, P], f32, tag="t2")
                        nc.vector.tensor_add(
                            out=t2[:, :tw],
                            in0=aggs[t][:, :tw],
                            in1=zT_sh[:, t * P : t * P + tw],
                        )
                        h2f = sm.tile([H, P], f16, tag="h2f")
                        nc.scalar.activation(
                            out=h2f[:, :tw], in_=t2[:, :tw], func=relu,
                            bias=b2_sb[:],
                        )
                        h3p = pst.tile([16, P], f32, tag="h3")
                        nc.tensor.matmul(
                            out=h3p[:, :tw], lhsT=w3_sb[:], rhs=h2f[:, :tw],
                            start=True, stop=True,
                        )
                        h3f = sm.tile([16, P], f16, tag="h3f")
                        nc.scalar.activation(
                            out=h3f[:, :tw], in_=h3p[:, :tw], func=relu,
                            bias=b3_sb[:],
                        )
                        op_ = pst.tile([1, P], f32, tag="op")
                        nc.tensor.matmul(
                            out=op_[:, :tw], lhsT=w4_sb[:], rhs=h3f[:, :tw],
                            start=True, stop=True,
                        )
                        nc.vector.scalar_tensor_tensor(
                            out=outT[:, t * P : t * P + tw],
                            in0=op_[:, :tw],
                            scalar=1.0,
                            in1=b4_sb[:].to_broadcast([1, tw]),
                            op0=mybir.AluOpType.mult,
                            op1=mybir.AluOpType.add,
                        )
            nc.sync.dma_start(out=out[:, :], in_=outT[:])
    nc.compile()
    return nc


def _in_maps(cfg, plan, x, W1, b1, W2, b2, W3, b3, W4, b4):
    c = cfg
    x16_a = x.astype(np.float16)
    iota_a = np.broadcast_to(
        np.arange(P, dtype=np.float16), (P, P)
    ).copy()
    ident_a = np.eye(c.H, dtype=np.float16)
    common = dict(
        x16=x16_a,
        iota=iota_a,
        ident=ident_a,
        w1=W1.astype(np.float16),
        w2=W2.astype(np.float16),
        w3=W3.astype(np.float16),
        w4=W4.astype(np.float16),
        b1=b1.reshape(-1, 1).astype(np.float32),
        b2=b2.reshape(-1, 1).astype(np.float32),
        b3=b3.reshape(-1, 1).astype(np.float32),
        b4v=b4.reshape(1, 1).astype(np.float32),
    )
    in_maps = []
    for ci in range(c.NCORE):
        idx16, dstloc = plan.core_arrays(ci)
        pk_a = np.concatenate([idx16, dstloc.view(np.int16)], axis=1)
        xT_a = np.ascontiguousarray(
            x[ci * c.SHARD : (ci + 1) * c.SHARD].T.astype(np.float32)
        )
        in_maps.append(dict(common, pk=pk_a, xT=xT_a))
    return in_maps


def _run(cfg, plan, nc, x, W1, b1, W2, b2, W3, b3, W4, b4, **kw):
    from concourse.bass_utils import run_bass_kernel_spmd

    c = cfg
    in_maps = _in_maps(cfg, plan, x, W1, b1, W2, b2, W3, b3, W4, b4)
    res = run_bass_kernel_spmd(nc, in_maps, core_ids=list(range(c.NCORE)), **kw)
    outs = [res.results[ci]["out"].reshape(-1) for ci in range(c.NCORE)]
    return np.concatenate(outs).reshape(-1, 1).astype(np.float32), res


def kernel(x, edge_index, W1, b1, W2, b2, W3, b3, W4, b4):
    cfg = FULL
    x = np.asarray(x, dtype=np.float32)
    src = np.asarray(edge_index[0], dtype=np.int64)
    dst = np.asarray(edge_index[1], dtype=np.int64)
    plan = Plan(cfg, src, dst)
    nc = _build(plan)
    out, _ = _run(
        cfg, plan, nc, x,
        np.asarray(W1), np.asarray(b1), np.asarray(W2), np.asarray(b2),
        np.asarray(W3), np.asarray(b3), np.asarray(W4), np.asarray(b4),
    )
    return out


# revision 10
# speedup vs baseline: 2.5604x; 2.5604x over previous
"""GIN message-passing (2 GINConv layers + 2 linear) on 8 TRN2 NeuronCores.

Strategy (self-contained; shapes hardcoded for the 100k-node / 1.6M-edge
problem):
  - Shard dst nodes across 8 cores (12500 each). Each core owns the edges
    whose dst is in its shard.
  - Per core, dst tiles of 128 nodes. For each tile, gather the source-node
    feature rows with gpsimd.dma_gather (int16 indices -> split the node
    table into 4 quarters of 25000 rows), then aggregate with a one-hot
    matmul on the tensor engine: aggT[ch, dst] += Xe[slots, ch].T @ onehot.
  - Layer 1 gathers x in fp16 (256B rows) straight from the replicated
    input; it computes z = relu((x + A@x)@W1 + b1) @ W2 per shard, writes z
    row-major fp32 to HBM, and an AllGather shares z across cores.
  - Layer 2 gathers z (fp32 256B rows), then h2 = relu(z_dst + A@z + b2),
    h3 = relu(h2@W3+b3), out = h3@W4+b4.
  - Gathers are batched over T_B dst tiles per call to amortize the ~1us
    SWDGE fixed cost; per-(tile,quarter) budgets are static (max over
    cores, rounded to 16) with idx-0 padding masked by dstloc=-1 one-hots.
"""

import numpy as np

P = 128


class Cfg:
    def __init__(self, n_nodes, n_edges, in_ch, hid, n_cores, t_b):
        assert n_nodes % (4 * n_cores) == 0
        self.N = n_nodes
        self.E = n_edges
        self.CH = in_ch
        self.H = hid
        self.NCORE = n_cores
        self.SHARD = n_nodes // n_cores
        self.NT = -(-self.SHARD // P)
        self.NQ = 4
        self.QS = n_nodes // 4
        assert self.QS <= 32768
        self.T_B = t_b
        self.NB = -(-self.NT // t_b)


FULL = Cfg(100000, 1600000, 128, 64, 8, 4)


def _r16(a):
    return ((a + 15) // 16) * 16


def _r128(a):
    return ((a + 127) // 128) * 128


class Plan:
    """Static (core-independent) schedule + per-core index arrays."""

    def __init__(self, cfg, src, dst):
        c = cfg
        self.cfg = c
        core = dst // c.SHARD
        # per-core grouped edge arrays
        self.counts = np.zeros((c.NCORE, c.NT, c.NQ), dtype=np.int64)
        self.groups = []  # [core] -> dict[(t,q)] -> (srclocal i16 array, dstlocal array)
        for ci in range(c.NCORE):
            m = core == ci
            s = src[m]
            d = dst[m] - ci * c.SHARD
            t = d // P
            q = s // c.QS
            key = t * c.NQ + q
            order = np.argsort(key, kind="stable")
            s, d, t, q, key = s[order], d[order], t[order], q[order], key[order]
            sl = (s % c.QS).astype(np.int16)
            dl = (d % P).astype(np.int16)
            cnt = np.bincount(key, minlength=c.NT * c.NQ).reshape(c.NT, c.NQ)
            self.counts[ci] = cnt
            bounds = np.concatenate([[0], np.cumsum(cnt.reshape(-1))])
            g = {}
            for tt in range(c.NT):
                for qq in range(c.NQ):
                    k = tt * c.NQ + qq
                    lo, hi = bounds[k], bounds[k + 1]
                    if hi > lo:
                        g[(tt, qq)] = (sl[lo:hi], dl[lo:hi])
            self.groups.append(g)

        # x128 so every tile's segment is chunk-aligned (PE matmuls need
        # base partition 0) and every matmul has full K=128
        self.B = _r128(self.counts.max(axis=0))  # [NT, NQ] slot budgets
        # per (batch, quarter): total slots, chunk layout, segments
        self.batches = []
        cgo = 0  # global chunk offset (dstloc columns)
        igo = 0  # global idx16 column offset
        for b in range(c.NB):
            tiles = list(range(b * c.T_B, min((b + 1) * c.T_B, c.NT)))
            quarters = []
            o = 0  # chunk offset within batch
            for q in range(c.NQ):
                segs = []  # (tile, chunk_in_q, p0, p1, slot0_of_tile_seg)
                s0 = 0
                for t in tiles:
                    btq = int(self.B[t, q])
                    if btq == 0:
                        continue
                    lo, hi = s0, s0 + btq
                    c0, c1 = lo // P, (hi - 1) // P
                    for ch in range(c0, c1 + 1):
                        p0 = max(lo, ch * P) - ch * P
                        p1 = min(hi, (ch + 1) * P) - ch * P
                        segs.append((t, ch, p0, p1))
                    s0 = hi
                tot = s0  # multiple of 16
                nch = -(-tot // P) if tot else 0
                quarters.append(
                    dict(tot=tot, nch=nch, o=o, icols=tot // 16, segs=segs)
                )
                o += nch
            self.batches.append(
                dict(tiles=tiles, quarters=quarters, cgo=cgo, igo=igo, nch=o)
            )
            cgo += o
            igo += sum(qq["icols"] for qq in quarters)
        self.CGOT = cgo
        self.IGOT = igo

    def core_arrays(self, ci):
        """idx16 [128, IGOT] int16 and dstloc [128, CGOT] f16 for core ci."""
        c = self.cfg
        idx16 = np.zeros((P, self.IGOT), dtype=np.int16)
        dstloc = np.full((P, self.CGOT), -1.0, dtype=np.float16)
        g = self.groups[ci]
        for b in self.batches:
            icol = b["igo"]
            for q, qq in enumerate(b["quarters"]):
                tot = qq["tot"]
                if tot == 0:
                    continue
                sl_full = np.zeros(tot, dtype=np.int16)
                dl_full = np.full(tot, -1, dtype=np.int16)
                s0 = 0
                for t in b["tiles"]:
                    btq = int(self.B[t, q])
                    if btq == 0:
                        continue
                    if (t, q) in g:
                        sl, dl = g[(t, q)]
                        n = len(sl)
                        sl_full[s0 : s0 + n] = sl
                        dl_full[s0 : s0 + n] = dl
                    s0 += btq
                # idx wrap-16 layout, replicated to 128 partitions
                blk = sl_full.reshape(tot // 16, 16).T  # [16, tot/16]
                idx16[:, icol : icol + tot // 16] = np.tile(blk, (8, 1))
                icol += tot // 16
                # dstloc slot k -> [k%128, cgo + o + k//128]
                col0 = b["cgo"] + qq["o"]
                nch = qq["nch"]
                dpad = np.full(nch * P, -1, dtype=np.int16)
                dpad[:tot] = dl_full
                dstloc[:, col0 : col0 + nch] = (
                    dpad.reshape(nch, P).T.astype(np.float16)
                )
        return idx16, dstloc


def _build(plan):
    import concourse.bass as bass
    import concourse.tile as tile
    from concourse import bacc, mybir

    c = plan.cfg
    f16, f32, i16 = mybir.dt.float16, mybir.dt.float32, mybir.dt.int16
    CH, H, NT, NQ, QS, SHARD = c.CH, c.H, c.NT, c.NQ, c.QS, c.SHARD

    nc = bacc.Bacc(
        "TRN2", target_bir_lowering=False, debug=False, num_devices=c.NCORE,
        num_swdge_queues=4,
    )
    x16 = nc.dram_tensor("x16", [c.N, CH], f16, kind="ExternalInput")
    xT = nc.dram_tensor("xT", [CH, SHARD], f32, kind="ExternalInput")
    pk = nc.dram_tensor(
        "pk", [P, plan.IGOT + plan.CGOT], i16, kind="ExternalInput"
    )
    iota = nc.dram_tensor("iota", [P, P], f16, kind="ExternalInput")
    ident = nc.dram_tensor("ident", [H, H], f16, kind="ExternalInput")
    w1 = nc.dram_tensor("w1", [CH, H], f16, kind="ExternalInput")
    w2 = nc.dram_tensor("w2", [H, H], f16, kind="ExternalInput")
    w3 = nc.dram_tensor("w3", [H, 16], f16, kind="ExternalInput")
    w4 = nc.dram_tensor("w4", [16, 1], f16, kind="ExternalInput")
    b1 = nc.dram_tensor("b1", [H, 1], f32, kind="ExternalInput")
    b2 = nc.dram_tensor("b2", [H, 1], f32, kind="ExternalInput")
    b3 = nc.dram_tensor("b3", [16, 1], f32, kind="ExternalInput")
    b4v = nc.dram_tensor("b4v", [1, 1], f32, kind="ExternalInput")
    out = nc.dram_tensor("out", [1, SHARD], f32, kind="ExternalOutput")

    with tile.TileContext(nc) as tc:
        with (
            tc.tile_pool(name="const", bufs=1) as cp,
            tc.tile_pool(name="persist", bufs=1) as pp,
            tc.tile_pool(name="dram", bufs=1, space="DRAM") as dp,
        ):
            iota_sb = cp.tile([P, P], f16)
            nc.sync.dma_start(out=iota_sb[:], in_=iota[:, :])
            id_sb = cp.tile([H, H], f16)
            nc.sync.dma_start(out=id_sb[:], in_=ident[:, :])
            w1_sb = cp.tile([CH, H], f16)
            nc.sync.dma_start(out=w1_sb[:], in_=w1[:, :])
            w2_sb = cp.tile([H, H], f16)
            nc.sync.dma_start(out=w2_sb[:], in_=w2[:, :])
            w3_sb = cp.tile([H, 16], f16)
            nc.sync.dma_start(out=w3_sb[:], in_=w3[:, :])
            w4_sb = cp.tile([16, 1], f16)
            nc.sync.dma_start(out=w4_sb[:], in_=w4[:, :])
            b1_sb = cp.tile([H, 1], f32)
            nc.sync.dma_start(out=b1_sb[:], in_=b1[:, :])
            b2_sb = cp.tile([H, 1], f32)
            nc.sync.dma_start(out=b2_sb[:], in_=b2[:, :])
            b3_sb = cp.tile([16, 1], f32)
            nc.sync.dma_start(out=b3_sb[:], in_=b3[:, :])
            b4_sb = cp.tile([1, 1], f32)
            nc.sync.dma_start(out=b4_sb[:], in_=b4v[:, :])

            zT_sh = pp.tile([H, SHARD], f32)  # identity term for layer 2
            outT = pp.tile([1, SHARD], f32)

            z_shard = dp.tile([SHARD, P], f16)
            z_full = dp.tile([c.N, P], f16)

            relu = mybir.ActivationFunctionType.Relu
            qrr = [0]  # round-robin SWDGE queue so gathers use all 4 Q7 pairs

            def tile_cols(t):
                return min(P, SHARD - t * P)

            # ---------------- phase 1 ----------------
            with (
                tc.tile_pool(name="l1g", bufs=2) as gp,
                tc.tile_pool(name="l1oh", bufs=2) as ohp,
                tc.tile_pool(name="l1sm", bufs=3) as sm,
                tc.tile_pool(name="l1ps", bufs=c.T_B + 1, space="PSUM") as psa,
                tc.tile_pool(name="l1pst", bufs=1, space="PSUM") as pst,
            ):
                for b in plan.batches:
                    nch = b["nch"]
                    if nch == 0:
                        continue
                    icols = sum(q["icols"] for q in b["quarters"])
                    pk_sb = sm.tile([P, icols + nch], i16, tag="pk")
                    nc.sync.dma_start(
                        out=pk_sb[:, :icols],
                        in_=pk[:, b["igo"] : b["igo"] + icols],
                    )
                    nc.sync.dma_start(
                        out=pk_sb[:, icols:],
                        in_=pk[
                            :,
                            plan.IGOT + b["cgo"] : plan.IGOT + b["cgo"] + nch,
                        ],
                    )
                    dl_sb = pk_sb[:, icols:].bitcast(f16)
                    g1 = gp.tile([P, nch, CH], f16, tag="g1")
                    ic = 0
                    for q, qq in enumerate(b["quarters"]):
                        if qq["tot"] == 0:
                            continue
                        # dma_gather is limited to 1024 indices per call
                        for s0 in range(0, qq["tot"], 1024):
                            n = min(1024, qq["tot"] - s0)
                            c0 = qq["o"] + s0 // P
                            nc.gpsimd.dma_gather(
                                g1[:, c0 : c0 + n // P, :],
                                x16[q * QS : (q + 1) * QS, :],
                                pk_sb[:, ic + s0 // 16 : ic + (s0 + n) // 16],
                                n,
                                n,
                                CH,
                                queue_num=qrr[0] % 4,
                            )
                            qrr[0] += 1
                        ic += qq["icols"]
                    oh = ohp.tile([P, nch, P], f16, tag="oh")
                    nc.vector.tensor_tensor(
                        out=oh[:],
                        in0=dl_sb.unsqueeze(2).to_broadcast([P, nch, P]),
                        in1=iota_sb[:].unsqueeze(1).to_broadcast([P, nch, P]),
                        op=mybir.AluOpType.is_equal,
                    )
                    aggs = {}
                    mm = []
                    for q, qq in enumerate(b["quarters"]):
                        for (t, ch, p0, p1) in qq["segs"]:
                            mm.append((t, qq["o"] + ch, p0, p1))
                    first = {}
                    last = {}
                    for i, (t, ch, p0, p1) in enumerate(mm):
                        first.setdefault(t, i)
                        last[t] = i
                    for t in b["tiles"]:
                        aggs[t] = psa.tile([CH, P], f32, tag="agg1", name=f"agg1_{t}")
                    for i, (t, ch, p0, p1) in enumerate(mm):
                        nc.tensor.matmul(
                            out=aggs[t][:],
                            lhsT=g1[p0:p1, ch : ch + 1, :],
                            rhs=oh[p0:p1, ch : ch + 1, :],
                            start=(i == first[t]),
                            stop=(i == last[t]),
                        )
                    for t in b["tiles"]:
                        tw = tile_cols(t)
                        xT_sb = sm.tile([CH, P], f32, tag="xt")
                        nc.scalar.dma_start(
                            out=xT_sb[:, :tw], in_=xT[:, t * P : t * P + tw]
                        )
                        sT = sm.tile([CH, P], f16, tag="st")
                        nc.vector.tensor_add(
                            out=sT[:, :tw],
                            in0=aggs[t][:, :tw],
                            in1=xT_sb[:, :tw],
                        )
                        h1p = pst.tile([H, P], f32, tag="h1")
                        nc.tensor.matmul(
                            out=h1p[:, :tw], lhsT=w1_sb[:], rhs=sT[:, :tw],
                            start=True, stop=True,
                        )
                        h1f = sm.tile([H, P], f16, tag="h1f")
                        nc.scalar.activation(
                            out=h1f[:, :tw], in_=h1p[:, :tw], func=relu,
                            bias=b1_sb[:],
                        )
                        zp = pst.tile([H, P], f32, tag="zp")
                        nc.tensor.matmul(
                            out=zp[:, :tw], lhsT=w2_sb[:], rhs=h1f[:, :tw],
                            start=True, stop=True,
                        )
                        nc.vector.tensor_copy(
                            out=zT_sh[:, t * P : t * P + tw], in_=zp[:, :tw]
                        )
                        zf = sm.tile([H, P], f16, tag="zf")
                        nc.vector.tensor_copy(out=zf[:, :tw], in_=zp[:, :tw])
                        ztr = pst.tile([P, H], f16, tag="ztr")
                        nc.tensor.transpose(
                            out=ztr[:tw, :], in_=zf[:, :tw], identity=id_sb[:]
                        )
                        zr = sm.tile([P, P], f16, tag="zr")
                        nc.vector.memset(zr[:, H:], 0)
                        nc.vector.tensor_copy(out=zr[:tw, :H], in_=ztr[:tw, :])
                        nc.scalar.dma_start(
                            out=z_shard[t * P : t * P + tw, :], in_=zr[:tw, :]
                        )

            nc.gpsimd.collective_compute(
                "AllGather",
                mybir.AluOpType.bypass,
                replica_groups=[list(range(c.NCORE))],
                ins=[z_shard[:].opt()],
                outs=[z_full[:].opt()],
            )

            # ---------------- phase 2 ----------------
            with (
                tc.tile_pool(name="l2g", bufs=2) as gp,
                tc.tile_pool(name="l2oh", bufs=2) as ohp,
                tc.tile_pool(name="l2sm", bufs=3) as sm,
                tc.tile_pool(name="l2ps", bufs=c.T_B + 1, space="PSUM") as psa,
                tc.tile_pool(name="l2pst", bufs=1, space="PSUM") as pst,
            ):
                for b in plan.batches:
                    nch = b["nch"]
                    if nch == 0:
                        continue
                    icols = sum(q["icols"] for q in b["quarters"])
                    pk_sb = sm.tile([P, icols + nch], i16, tag="pk")
                    nc.sync.dma_start(
                        out=pk_sb[:, :icols],
                        in_=pk[:, b["igo"] : b["igo"] + icols],
                    )
                    nc.sync.dma_start(
                        out=pk_sb[:, icols:],
                        in_=pk[
                            :,
                            plan.IGOT + b["cgo"] : plan.IGOT + b["cgo"] + nch,
                        ],
                    )
                    dl_sb = pk_sb[:, icols:].bitcast(f16)
                    g2f = gp.tile([P, nch, P], f16, tag="g2f")
                    ic = 0
                    for q, qq in enumerate(b["quarters"]):
                        if qq["tot"] == 0:
                            continue
                        for s0 in range(0, qq["tot"], 1024):
                            n = min(1024, qq["tot"] - s0)
                            c0 = qq["o"] + s0 // P
                            nc.gpsimd.dma_gather(
                                g2f[:, c0 : c0 + n // P, :],
                                z_full[q * QS : (q + 1) * QS, :],
                                pk_sb[:, ic + s0 // 16 : ic + (s0 + n) // 16],
                                n,
                                n,
                                P,
                                queue_num=qrr[0] % 4,
                            )
                            qrr[0] += 1
                        ic += qq["icols"]
                    oh = ohp.tile([P, nch, P], f16, tag="oh")
                    nc.vector.tensor_tensor(
                        out=oh[:],
                        in0=dl_sb.unsqueeze(2).to_broadcast([P, nch, P]),
                        in1=iota_sb[:].unsqueeze(1).to_broadcast([P, nch, P]),
                        op=mybir.AluOpType.is_equal,
                    )
                    mm = []
                    for q, qq in enumerate(b["quarters"]):
                        for (t, ch, p0, p1) in qq["segs"]:
                            mm.append((t, qq["o"] + ch, p0, p1))
                    first = {}
                    last = {}
                    for i, (t, ch, p0, p1) in enumerate(mm):
                        first.setdefault(t, i)
                        last[t] = i
                    aggs = {}
                    for t in b["tiles"]:
                        aggs[t] = psa.tile([P, P], f32, tag="agg2", name=f"agg2_{t}")
                    for i, (t, ch, p0, p1) in enumerate(mm):
                        nc.tensor.matmul(
                            out=aggs[t][:],
                            lhsT=g2f[p0:p1, ch : ch + 1, :],
                            rhs=oh[p0:p1, ch : ch + 1, :],
                            start=(i == first[t]),
                            stop=(i == last[t]),
                        )
                    for t in b["tiles"]:
                        tw = tile_cols(t)
                        t2 = sm.tile([H, P], f32, tag="t2")
                        nc.vector.tensor_add(
                            out=t2[:, :tw],
                            in0=aggs[t][:H, :tw],
                            in1=zT_sh[:, t * P : t * P + tw],
                        )
                        h2f = sm.tile([H, P], f16, tag="h2f")
                        nc.scalar.activation(
                            out=h2f[:, :tw], in_=t2[:, :tw], func=relu,
                            bias=b2_sb[:],
                        )
                        h3p = pst.tile([16, P], f32, tag="h3")
                        nc.tensor.matmul(
                            out=h3p[:, :tw], lhsT=w3_sb[:], rhs=h2f[:, :tw],
                            start=True, stop=True,
                        )
                        h3f = sm.tile([16, P], f16, tag="h3f")
                        nc.scalar.activation(
                            out=h3f[:, :tw], in_=h3p[:, :tw], func=relu,
                            bias=b3_sb[:],
                        )
                        op_ = pst.tile([1, P], f32, tag="op")
                        nc.tensor.matmul(
                            out=op_[:, :tw], lhsT=w4_sb[:], rhs=h3f[:, :tw],
                            start=True, stop=True,
                        )
                        nc.vector.scalar_tensor_tensor(
                            out=outT[:, t * P : t * P + tw],
                            in0=op_[:, :tw],
                            scalar=1.0,
                            in1=b4_sb[:].to_broadcast([1, tw]),
                            op0=mybir.AluOpType.mult,
                            op1=mybir.AluOpType.add,
                        )
            nc.sync.dma_start(out=out[:, :], in_=outT[:])
    nc.compile()
    return nc


def _in_maps(cfg, plan, x, W1, b1, W2, b2, W3, b3, W4, b4):
    c = cfg
    x16_a = x.astype(np.float16)
    iota_a = np.broadcast_to(
        np.arange(P, dtype=np.float16), (P, P)
    ).copy()
    ident_a = np.eye(c.H, dtype=np.float16)
    common = dict(
        x16=x16_a,
        iota=iota_a,
        ident=ident_a,
        w1=W1.astype(np.float16),
        w2=W2.astype(np.float16),
        w3=W3.astype(np.float16),
        w4=W4.astype(np.float16),
        b1=b1.reshape(-1, 1).astype(np.float32),
        b2=b2.reshape(-1, 1).astype(np.float32),
        b3=b3.reshape(-1, 1).astype(np.float32),
        b4v=b4.reshape(1, 1).astype(np.float32),
    )
    in_maps = []
    for ci in range(c.NCORE):
        idx16, dstloc = plan.core_arrays(ci)
        pk_a = np.concatenate([idx16, dstloc.view(np.int16)], axis=1)
        xT_a = np.ascontiguousarray(
            x[ci * c.SHARD : (ci + 1) * c.SHARD].T.astype(np.float32)
        )
        in_maps.append(dict(common, pk=pk_a, xT=xT_a))
    return in_maps


def _run(cfg, plan, nc, x, W1, b1, W2, b2, W3, b3, W4, b4, **kw):
    from concourse.bass_utils import run_bass_kernel_spmd

    c = cfg
    in_maps = _in_maps(cfg, plan, x, W1, b1, W2, b2, W3, b3, W4, b4)
    res = run_bass_kernel_spmd(nc, in_maps, core_ids=list(range(c.NCORE)), **kw)
    outs = [res.results[ci]["out"].reshape(-1) for ci in range(c.NCORE)]
    return np.concatenate(outs).reshape(-1, 1).astype(np.float32), res


def kernel(x, edge_index, W1, b1, W2, b2, W3, b3, W4, b4):
    cfg = FULL
    x = np.asarray(x, dtype=np.float32)
    src = np.asarray(edge_index[0], dtype=np.int64)
    dst = np.asarray(edge_index[1], dtype=np.int64)
    plan = Plan(cfg, src, dst)
    nc = _build(plan)
    out, _ = _run(
        cfg, plan, nc, x,
        np.asarray(W1), np.asarray(b1), np.asarray(W2), np.asarray(b2),
        np.asarray(W3), np.asarray(b3), np.asarray(W4), np.asarray(b4),
    )
    return out
